# revision 29
# baseline (speedup 1.0000x reference)
# Paged sparse attention (GQA, block-masked new tokens) on 8 TRN2 NeuronCores.
#
# Sharding: tensor-parallel over the 8 KV heads (one KV head + its 4 Q heads
# per core). Every core sees all 8 sequences, so the compiled schedule
# (derived from page_tables/context_lens, identical across cores) is SPMD.
#
# Orientation: scores are computed TRANSPOSED (S^T[t, sg] per 128-row
# t-block, K^T-stationary, Q^T-moving), so the exp (ACT) writes P^T directly
# in the layout the PV matmul consumes — no probability transposes anywhere.
# Masking folds into the exp's per-partition bias (host-precomputed -1e30
# rows for the partial page / 32-alignment gap / tail pad). The softmax
# denominator comes from an extra matmul with an all-ones stationary matrix,
# which leaves the per-sg denominator replicated across all 128 PSUM
# partitions — the normalization is then a single fused
# (OUT^T * 1/denom -> bf16) DVE pass, transposed back to [sg, d] by one
# DMA-xbar call per sequence.
#
# The block-causal mask for new tokens reduces (with sg = s*4+g ordering) to
# a suffix of valid sg columns per t-block (plus a small intra-block
# staircase zeroed on the bf16 P^T), so invalid regions are simply never
# computed.

import sys

sys.path.insert(0, "/opt/trn_rl_repo")

import ml_dtypes
import numpy as np

B = 8
S = 256
NUM_HEADS = 32
NUM_KV_HEADS = 8
G = NUM_HEADS // NUM_KV_HEADS  # 4
HD = 128
PAGE = 16
BLOCK = 32
MAX_PAGES = 128
C = MAX_PAGES * PAGE  # 2048
SCALE = 0.08838834764831845
SG = S * G  # 1024 q rows per (seq, kv head)
TMAX = C + S + 32  # worst-case padded length
NTBMAX = (TMAX + 127) // 128
NQT = SG // 128  # 8 q-tiles per seq

NEG = -1e30


def _schedule(page_tables: np.ndarray, context_lens: np.ndarray):
    """Per-seq schedule baked into the compiled kernel (same on all cores)."""
    seqs = []
    for b in range(B):
        ctx = int(context_lens[b])
        npg = (ctx + PAGE - 1) // PAGE
        ctxp = npg * PAGE
        ctxp32 = ((ctxp + 31) // 32) * 32  # 32-align the new-token region
        pages = [int(p) for p in page_tables[b, :npg]]
        runs = []  # maximal consecutive-page runs -> [start_page, num_pages]
        for p in pages:
            if runs and runs[-1][0] + runs[-1][1] == p:
                runs[-1][1] += 1
            else:
                runs.append([p, 1])
        ttot = ctxp32 + S
        ntb = (ttot + 127) // 128
        tq = [ctxp32 + BLOCK * (i + 1) for i in range(NQT)]
        # first valid q-tile per t-block (valid sg columns = suffix)
        qmin = [next(i for i in range(NQT) if tq[i] > tb * 128) for tb in range(ntb)]

        def fully_valid(tb):
            # every t-row in the block is a real, unmasked token
            if (tb + 1) * 128 > ttot:
                return False
            return not (ctx < (tb + 1) * 128 and tb * 128 < ctxp32)

        # one exp call per t-block (pairing loses more on PSUM
        # double-buffering than it saves on ACT per-call overhead)
        groups = [(tb, 1) for tb in range(ntb)]
        seqs.append(
            dict(
                ctx=ctx,
                ctxp=ctxp,
                ctxp32=ctxp32,
                runs=runs,
                ttot=ttot,
                ntb=ntb,
                tq=tq,
                qmin=qmin,
                groups=groups,
                valid=[fully_valid(tb) for tb in range(ntb)],
            )
        )
    return seqs


def _masks(seqs):
    """Host-precomputed per-partition exp bias: [B, 128, NTBMAX] fp32.
    mask[b, p, tb] is added (post-scale) to scores of t-row tb*128+p:
    0 for valid rows, -1e30 for masked rows (partial page, 32-align gap,
    padded tail)."""
    m = np.zeros((B, 128, NTBMAX), np.float32)
    for b, sq in enumerate(seqs):
        valid = np.zeros((NTBMAX * 128,), bool)
        valid[: sq["ttot"]] = True
        valid[sq["ctx"] : sq["ctxp32"]] = False  # partial page + gap
        m[b][~valid.reshape(NTBMAX, 128).T] = NEG
    return m


def _build(nc, seqs):
    import concourse.mybir as mybir
    import concourse.tile as tile

    bf16 = mybir.dt.bfloat16
    f32 = mybir.dt.float32

    qh = nc.dram_tensor("qh", [B * S, G * HD], bf16, kind="ExternalInput").ap()
    kh = nc.dram_tensor("kh", [B * S, HD], bf16, kind="ExternalInput").ap()
    vh = nc.dram_tensor("vh", [B * S, HD], bf16, kind="ExternalInput").ap()
    kch = nc.dram_tensor("kch", [MAX_PAGES * B * PAGE, HD], bf16, kind="ExternalInput").ap()
    vch = nc.dram_tensor("vch", [MAX_PAGES * B * PAGE, HD], bf16, kind="ExternalInput").ap()
    mh = nc.dram_tensor("mh", [B, 128, NTBMAX], f32, kind="ExternalInput").ap()
    zz = nc.dram_tensor("zz", [32, HD], bf16, kind="ExternalInput").ap()
    # transposed output [b, d, sg]; the host reindexes during the gather
    outh = nc.dram_tensor("outh", [B, HD, SG], f32, kind="ExternalOutput").ap()

    # q viewed per seq as [sg=(s,g), d]; contiguous because each q row holds
    # the 4 grouped heads back to back.
    qv = qh.rearrange("(b s) (g d) -> b (s g) d", b=B, d=HD)

    with tile.TileContext(nc) as tc:
        with (
            tc.tile_pool(name="cst", bufs=1) as const_pool,
            tc.tile_pool(name="kt", bufs=3) as kt_pool,
            tc.tile_pool(name="vt", bufs=3) as v_pool,
            tc.tile_pool(name="qt", bufs=3) as qt_pool,
            tc.tile_pool(name="pt", bufs=2) as pt_pool,
            tc.tile_pool(name="mk", bufs=3) as mask_pool,
            tc.tile_pool(name="ot", bufs=2) as out_pool,
            tc.tile_pool(name="ps_s", bufs=2, space="PSUM") as psum_s,
            tc.tile_pool(name="ps_o", bufs=1, space="PSUM") as psum_o,
            tc.tile_pool(name="ps_d", bufs=1, space="PSUM") as psum_d,
        ):
            ones_t = const_pool.tile([128, 128], bf16)
            nc.vector.memset(ones_t, 1.0)
            # all seqs' exp bias masks in one load: [128, b, ntb]
            mask_all = const_pool.tile([128, B, NTBMAX], f32)
            nc.sync.dma_start(mask_all, mh.rearrange("b p n -> p b n"))

            tiles = {}

            def emit_loads(b):
                sq = seqs[b]
                ctx, ctxp, ctxp32 = sq["ctx"], sq["ctxp"], sq["ctxp32"]
                ttot, ntb = sq["ttot"], sq["ntb"]

                # K^T via transposed loads, split into two tiles so the
                # first t-blocks' matmuls start before the full cache lands
                KSPLIT = 1024
                kta = kt_pool.tile([128, KSPLIT], bf16, tag="kta")
                ktb = kt_pool.tile([128, NTBMAX * 128 - KSPLIT], bf16, tag="ktb")

                def kt_slice(c0, c1):
                    if c0 >= KSPLIT:
                        return ktb[:, c0 - KSPLIT : c1 - KSPLIT]
                    return kta[:, c0:c1]

                def kt_load_transpose(c0, rows, src):
                    # split a transposed load at the tile boundary
                    if c0 < KSPLIT < c0 + rows:
                        nc.sync.dma_start_transpose(
                            kta[:, c0:KSPLIT], src[: KSPLIT - c0, :]
                        )
                        nc.sync.dma_start_transpose(
                            ktb[:, : c0 + rows - KSPLIT], src[KSPLIT - c0 :, :]
                        )
                    else:
                        nc.sync.dma_start_transpose(kt_slice(c0, c0 + rows), src)

                def kt_memset(c0, c1):
                    if c0 < KSPLIT < c1:
                        nc.vector.memset(kta[:, c0:KSPLIT], 0.0)
                        nc.vector.memset(ktb[:, : c1 - KSPLIT], 0.0)
                    else:
                        nc.vector.memset(kt_slice(c0, c1), 0.0)

                col = 0
                for start, n in sq["runs"]:
                    kt_load_transpose(
                        col, n * PAGE, kch[start * PAGE : (start + n) * PAGE, :]
                    )
                    col += n * PAGE
                assert col == ctxp
                if ctxp32 > ctxp:  # 32-align gap: zero K columns
                    kt_memset(ctxp, ctxp32)
                kt_load_transpose(ctxp32, S, kh[b * S : (b + 1) * S, :])
                if ntb * 128 > ttot:  # zero padded tail columns
                    kt_memset(ttot, ntb * 128)

                # V natural [t%128, tb, d]; big rearranged DMAs
                vt = v_pool.tile([128, NTBMAX, HD], bf16, tag="vt")
                if ttot % 128:
                    # zero last block before loads (NaN-safe padded tail)
                    nc.vector.memset(vt[:, ntb - 1, :], 0.0)

                def load_v_rows(t0, nrows, src, src_row0):
                    while nrows > 0 and t0 % 128:
                        seg = min(nrows, 128 - t0 % 128)
                        nc.gpsimd.dma_start(
                            vt[t0 % 128 : t0 % 128 + seg, t0 // 128, :],
                            src[src_row0 : src_row0 + seg, :],
                        )
                        t0 += seg
                        src_row0 += seg
                        nrows -= seg
                    nfull = (nrows // 128) * 128
                    if nfull:
                        nc.gpsimd.dma_start(
                            vt[:, t0 // 128 : t0 // 128 + nfull // 128, :],
                            src[src_row0 : src_row0 + nfull, :].rearrange(
                                "(tb p) d -> p tb d", p=128
                            ),
                        )
                        t0 += nfull
                        src_row0 += nfull
                        nrows -= nfull
                    if nrows:
                        nc.gpsimd.dma_start(
                            vt[:nrows, t0 // 128, :],
                            src[src_row0 : src_row0 + nrows, :],
                        )

                col = 0
                for start, n in sq["runs"]:
                    load_v_rows(col, n * PAGE, vch, start * PAGE)
                    col += n * PAGE
                if ctxp32 > ctxp and ctxp // 128 != ntb - 1:
                    # NaN-safe zeros for the gap rows
                    load_v_rows(ctxp, ctxp32 - ctxp, zz, 0)
                load_v_rows(ctxp32, S, vh, b * S)

                # Q^T via one transposed load
                qt = qt_pool.tile([128, SG], bf16, tag="qt")
                nc.sync.dma_start_transpose(qt, qv[b])
                tiles[b] = ((kta, ktb), vt, qt, mask_all[:, b, :])

            def emit_compute(b):
                sq = seqs[b]
                ctxp32, ttot, ntb = sq["ctxp32"], sq["ttot"], sq["ntb"]
                tq, qmin = sq["tq"], sq["qmin"]
                (kta, ktb), vt, qt, mask_sb = tiles[b]

                # Interleaved per t-block: scores(tb) on PE while exp(tb-1)
                # runs on ACT, then PV(tb-1)+denom(tb-1) right behind it.
                ptt = pt_pool.tile([128, NTBMAX, SG], bf16, tag="pt")
                outt = psum_o.tile([128, SG], f32, tag="outt")
                dent = psum_d.tile([128, SG], f32, tag="dent")
                chunks = ((0, 4), (4, 8))
                last_tb = [0, 0]
                for tb in range(ntb):
                    for ci, (g0, g1) in enumerate(chunks):
                        if max(qmin[tb], g0) < g1:
                            last_tb[ci] = tb

                def emit_scores(tb0, ng):
                    # ng t-blocks (1 or 2) share one psum tile + one exp call
                    qm = qmin[tb0]
                    s_ps = psum_s.tile([128, ng * SG], f32, tag="s")
                    for j in range(ng):
                        for c0, c1 in ((qm * 128, 512), (max(512, qm * 128), SG)):
                            if c0 >= c1:
                                continue
                            tb = tb0 + j
                            lt = (
                                kta[:, tb * 128 : (tb + 1) * 128]
                                if tb < 8
                                else ktb[:, tb * 128 - 1024 : (tb + 1) * 128 - 1024]
                            )
                            nc.tensor.matmul(
                                s_ps[:, j * SG + c0 : j * SG + c1],
                                lhsT=lt,
                                rhs=qt[:, c0:c1],
                                start=True,
                                stop=True,
                            )
                    if ng == 2:
                        assert qm == 0
                        nc.scalar.activation(
                            out=ptt[:, tb0 : tb0 + 2, :],
                            in_=s_ps,
                            func=mybir.ActivationFunctionType.Exp,
                            scale=SCALE,
                        )
                    else:
                        nc.scalar.activation(
                            out=ptt[:, tb0, qm * 128 :],
                            in_=s_ps[:, qm * 128 : SG],
                            func=mybir.ActivationFunctionType.Exp,
                            scale=SCALE,
                            bias=(
                                0.0
                                if sq["valid"][tb0]
                                else mask_sb[:, tb0 : tb0 + 1]
                            ),
                        )
                    # staircase: zero P^T rows of new-token blocks for
                    # earlier q-tiles inside this t-block's suffix
                    for tb in range(tb0, tb0 + ng):
                        for r0 in range(0, 128, 32):
                            t0 = tb * 128 + r0
                            if t0 < ctxp32 or t0 >= ttot:
                                continue
                            blk = (t0 - ctxp32) // 32
                            if blk > qmin[tb]:
                                nc.vector.memset(
                                    ptt[
                                        r0 : r0 + 32, tb, qmin[tb] * 128 : blk * 128
                                    ],
                                    0.0,
                                )

                def emit_pv(tb):
                    for ci, (g0, g1) in enumerate(chunks):
                        lo = max(qmin[tb], g0)
                        if lo >= g1:
                            continue
                        nc.tensor.matmul(
                            outt[:, lo * 128 : g1 * 128],
                            lhsT=vt[:, tb, :],
                            rhs=ptt[:, tb, lo * 128 : g1 * 128],
                            start=(tb == 0),
                            stop=(tb == last_tb[ci]),
                        )
                    for ci, (g0, g1) in enumerate(chunks):
                        lo = max(qmin[tb], g0)
                        if lo >= g1:
                            continue
                        nc.tensor.matmul(
                            dent[:, lo * 128 : g1 * 128],
                            lhsT=ones_t,
                            rhs=ptt[:, tb, lo * 128 : g1 * 128],
                            start=(tb == 0),
                            stop=(tb == last_tb[ci]),
                        )

                pending = []
                for tb0, ng in sq["groups"]:
                    emit_scores(tb0, ng)
                    for tb in pending:
                        emit_pv(tb)
                    pending = list(range(tb0, tb0 + ng))
                for tb in pending:
                    emit_pv(tb)
                tiles[b] = (outt, dent)

            def emit_endgame(b):
                outt, dent = tiles.pop(b)
                # OUT^T * (1/denom) -> fp32 -> HBM (host reindexes [d,sg])
                invt = out_pool.tile([128, SG], f32, tag="invt")
                nc.vector.reciprocal_approx_fast(invt, dent)
                otf = out_pool.tile([128, SG], f32, tag="otf")
                nc.vector.tensor_mul(otf, outt, invt)
                nc.gpsimd.dma_start(outh[b], otf)

            # software-pipelined emission: the in-order SP/Pool sequencers
            # must issue seq b+2's loads before blocking on seq b's endgame.
            # Process largest seqs first: their long compute covers the
            # load latency of everything behind them.
            order = sorted(range(B), key=lambda b: -seqs[b]["ntb"])
            emit_loads(order[0])
            emit_loads(order[1])
            for j, b in enumerate(order):
                emit_compute(b)
                if j + 2 < B:
                    emit_loads(order[j + 2])
                emit_endgame(b)
    return nc


def _compile(seqs):
    import concourse.bacc as bacc

    nc = bacc.Bacc(
        "TRN2",
        target_bir_lowering=False,
        debug=False,
        enable_asserts=False,
        num_devices=8,
    )
    _build(nc, seqs)
    nc.compile()
    return nc


def kernel(q, k, v, k_cache, v_cache, page_tables, context_lens, page_size, block_size, **_):
    from concourse import bass_utils

    q = np.asarray(q)
    k = np.asarray(k)
    v = np.asarray(v)
    k_cache = np.asarray(k_cache)
    v_cache = np.asarray(v_cache)
    page_tables = np.asarray(page_tables)
    context_lens = np.asarray(context_lens)
    assert int(page_size) == PAGE and int(block_size) == BLOCK
    assert q.shape == (B * S, NUM_HEADS * HD)
    assert page_tables.shape == (B, MAX_PAGES)

    seqs = _schedule(page_tables, context_lens)
    nc = _compile(seqs)

    bf = ml_dtypes.bfloat16
    masks = _masks(seqs)
    kcv = k_cache.reshape(MAX_PAGES * B * PAGE, NUM_KV_HEADS, HD)
    vcv = v_cache.reshape(MAX_PAGES * B * PAGE, NUM_KV_HEADS, HD)
    zz = np.zeros((32, HD), bf)
    in_maps = []
    for n in range(NUM_KV_HEADS):
        in_maps.append(
            {
                "qh": np.ascontiguousarray(
                    q[:, n * G * HD : (n + 1) * G * HD]
                ).astype(bf),
                "kh": np.ascontiguousarray(k[:, n * HD : (n + 1) * HD]).astype(bf),
                "vh": np.ascontiguousarray(v[:, n * HD : (n + 1) * HD]).astype(bf),
                "kch": np.ascontiguousarray(kcv[:, n, :]).astype(bf),
                "vch": np.ascontiguousarray(vcv[:, n, :]).astype(bf),
                "mh": masks,
                "zz": zz,
            }
        )

    res = bass_utils.run_bass_kernel_spmd(nc, in_maps, core_ids=list(range(8)))
    global _last_results
    _last_results = res
    # per-core outh is [B, HD, SG=(s,g)]; assemble [B*S, (n,g)*HD]
    out = np.empty((B * S, NUM_HEADS * HD), np.float32)
    ov = out.reshape(B, S, NUM_KV_HEADS, G, HD)
    for n in range(NUM_KV_HEADS):
        # [B, HD, S*G] -> [B, S, G, HD]
        on = res.results[n]["outh"].reshape(B, HD, S, G)
        ov[:, :, n, :, :] = on.transpose(0, 2, 3, 1)
    return out


_last_results = None


# revision 31
# speedup vs baseline: 1.0149x; 1.0149x over previous
# Paged sparse attention (GQA, block-masked new tokens) on 8 TRN2 NeuronCores.
#
# Sharding: tensor-parallel over the 8 KV heads (one KV head + its 4 Q heads
# per core). Every core sees all 8 sequences, so the compiled schedule
# (derived from page_tables/context_lens, identical across cores) is SPMD.
#
# Orientation: scores are computed TRANSPOSED (S^T[t, sg] per 128-row
# t-block, K^T-stationary, Q^T-moving), so the exp (ACT) writes P^T directly
# in the layout the PV matmul consumes — no probability transposes anywhere.
# Masking folds into the exp's per-partition bias (host-precomputed -1e30
# rows for the partial page / 32-alignment gap / tail pad). The softmax
# denominator comes from an extra matmul with an all-ones stationary matrix,
# which leaves the per-sg denominator replicated across all 128 PSUM
# partitions — the normalization is then a single fused
# (OUT^T * 1/denom -> bf16) DVE pass, transposed back to [sg, d] by one
# DMA-xbar call per sequence.
#
# The block-causal mask for new tokens reduces (with sg = s*4+g ordering) to
# a suffix of valid sg columns per t-block (plus a small intra-block
# staircase zeroed on the bf16 P^T), so invalid regions are simply never
# computed.

import sys

sys.path.insert(0, "/opt/trn_rl_repo")

import ml_dtypes
import numpy as np

B = 8
S = 256
NUM_HEADS = 32
NUM_KV_HEADS = 8
G = NUM_HEADS // NUM_KV_HEADS  # 4
HD = 128
PAGE = 16
BLOCK = 32
MAX_PAGES = 128
C = MAX_PAGES * PAGE  # 2048
SCALE = 0.08838834764831845
SG = S * G  # 1024 q rows per (seq, kv head)
TMAX = C + S + 32  # worst-case padded length
NTBMAX = (TMAX + 127) // 128
NQT = SG // 128  # 8 q-tiles per seq

NEG = -1e30


def _schedule(page_tables: np.ndarray, context_lens: np.ndarray):
    """Per-seq schedule baked into the compiled kernel (same on all cores)."""
    seqs = []
    for b in range(B):
        ctx = int(context_lens[b])
        npg = (ctx + PAGE - 1) // PAGE
        ctxp = npg * PAGE
        ctxp32 = ((ctxp + 31) // 32) * 32  # 32-align the new-token region
        pages = [int(p) for p in page_tables[b, :npg]]
        runs = []  # maximal consecutive-page runs -> [start_page, num_pages]
        for p in pages:
            if runs and runs[-1][0] + runs[-1][1] == p:
                runs[-1][1] += 1
            else:
                runs.append([p, 1])
        ttot = ctxp32 + S
        ntb = (ttot + 127) // 128
        tq = [ctxp32 + BLOCK * (i + 1) for i in range(NQT)]
        # first valid q-tile per t-block (valid sg columns = suffix)
        qmin = [next(i for i in range(NQT) if tq[i] > tb * 128) for tb in range(ntb)]

        def fully_valid(tb):
            # every t-row in the block is a real, unmasked token
            if (tb + 1) * 128 > ttot:
                return False
            return not (ctx < (tb + 1) * 128 and tb * 128 < ctxp32)

        # one exp call per t-block (pairing loses more on PSUM
        # double-buffering than it saves on ACT per-call overhead)
        groups = [(tb, 1) for tb in range(ntb)]
        seqs.append(
            dict(
                ctx=ctx,
                ctxp=ctxp,
                ctxp32=ctxp32,
                runs=runs,
                ttot=ttot,
                ntb=ntb,
                tq=tq,
                qmin=qmin,
                groups=groups,
                valid=[fully_valid(tb) for tb in range(ntb)],
            )
        )
    return seqs


def _masks(seqs):
    """Host-precomputed per-partition exp bias: [B, 128, NTBMAX] fp32.
    mask[b, p, tb] is added (post-scale) to scores of t-row tb*128+p:
    0 for valid rows, -1e30 for masked rows (partial page, 32-align gap,
    padded tail)."""
    m = np.zeros((B, 128, NTBMAX), np.float32)
    for b, sq in enumerate(seqs):
        valid = np.zeros((NTBMAX * 128,), bool)
        valid[: sq["ttot"]] = True
        valid[sq["ctx"] : sq["ctxp32"]] = False  # partial page + gap
        m[b][~valid.reshape(NTBMAX, 128).T] = NEG
    return m


def _build(nc, seqs):
    import concourse.mybir as mybir
    import concourse.tile as tile

    bf16 = mybir.dt.bfloat16
    f32 = mybir.dt.float32

    qh = nc.dram_tensor("qh", [B * S, G * HD], bf16, kind="ExternalInput").ap()
    kh = nc.dram_tensor("kh", [B * S, HD], bf16, kind="ExternalInput").ap()
    vh = nc.dram_tensor("vh", [B * S, HD], bf16, kind="ExternalInput").ap()
    kch = nc.dram_tensor("kch", [MAX_PAGES * B * PAGE, HD], bf16, kind="ExternalInput").ap()
    vch = nc.dram_tensor("vch", [MAX_PAGES * B * PAGE, HD], bf16, kind="ExternalInput").ap()
    mh = nc.dram_tensor("mh", [B, 128, NTBMAX], f32, kind="ExternalInput").ap()
    zz = nc.dram_tensor("zz", [32, HD], bf16, kind="ExternalInput").ap()
    # transposed output [b, d, sg]; the host reindexes during the gather
    outh = nc.dram_tensor("outh", [B, HD, SG], f32, kind="ExternalOutput").ap()

    # q viewed per seq as [sg=(s,g), d]; contiguous because each q row holds
    # the 4 grouped heads back to back.
    qv = qh.rearrange("(b s) (g d) -> b (s g) d", b=B, d=HD)

    with tile.TileContext(nc) as tc:
        with (
            tc.tile_pool(name="cst", bufs=1) as const_pool,
            tc.tile_pool(name="kt", bufs=3) as kt_pool,
            tc.tile_pool(name="vt", bufs=3) as v_pool,
            tc.tile_pool(name="qt", bufs=3) as qt_pool,
            tc.tile_pool(name="pt", bufs=2) as pt_pool,
            tc.tile_pool(name="mk", bufs=3) as mask_pool,
            tc.tile_pool(name="ot", bufs=2) as out_pool,
            tc.tile_pool(name="ps_s", bufs=2, space="PSUM") as psum_s,
            tc.tile_pool(name="ps_o", bufs=1, space="PSUM") as psum_o,
            tc.tile_pool(name="ps_d", bufs=1, space="PSUM") as psum_d,
        ):
            ones_t = const_pool.tile([128, 128], bf16)
            nc.vector.memset(ones_t, 1.0)
            # all seqs' exp bias masks in one load: [128, b, ntb]
            mask_all = const_pool.tile([128, B, NTBMAX], f32)
            nc.sync.dma_start(mask_all, mh.rearrange("b p n -> p b n"))

            tiles = {}

            def emit_loads(b):
                sq = seqs[b]
                ctx, ctxp, ctxp32 = sq["ctx"], sq["ctxp"], sq["ctxp32"]
                ttot, ntb = sq["ttot"], sq["ntb"]

                # K^T via transposed loads, split into two tiles so the
                # first t-blocks' matmuls start before the full cache lands
                KSPLIT = 1024
                kta = kt_pool.tile([128, KSPLIT], bf16, tag="kta")
                ktb = kt_pool.tile([128, NTBMAX * 128 - KSPLIT], bf16, tag="ktb")

                def kt_slice(c0, c1):
                    if c0 >= KSPLIT:
                        return ktb[:, c0 - KSPLIT : c1 - KSPLIT]
                    return kta[:, c0:c1]

                def kt_load_transpose(c0, rows, src):
                    # split a transposed load at the tile boundary
                    if c0 < KSPLIT < c0 + rows:
                        nc.sync.dma_start_transpose(
                            kta[:, c0:KSPLIT], src[: KSPLIT - c0, :]
                        )
                        nc.sync.dma_start_transpose(
                            ktb[:, : c0 + rows - KSPLIT], src[KSPLIT - c0 :, :]
                        )
                    else:
                        nc.sync.dma_start_transpose(kt_slice(c0, c0 + rows), src)

                def kt_memset(c0, c1):
                    if c0 < KSPLIT < c1:
                        nc.vector.memset(kta[:, c0:KSPLIT], 0.0)
                        nc.vector.memset(ktb[:, : c1 - KSPLIT], 0.0)
                    else:
                        nc.vector.memset(kt_slice(c0, c1), 0.0)

                # Q^T first on the SP queue: the first score matmul needs
                # only qt + the first K^T tile
                qt = qt_pool.tile([128, SG], bf16, tag="qt")
                nc.sync.dma_start_transpose(qt, qv[b])

                col = 0
                for start, n in sq["runs"]:
                    kt_load_transpose(
                        col, n * PAGE, kch[start * PAGE : (start + n) * PAGE, :]
                    )
                    col += n * PAGE
                assert col == ctxp
                if ctxp32 > ctxp:  # 32-align gap: zero K columns
                    kt_memset(ctxp, ctxp32)
                kt_load_transpose(ctxp32, S, kh[b * S : (b + 1) * S, :])
                if ntb * 128 > ttot:  # zero padded tail columns
                    kt_memset(ttot, ntb * 128)

                # V natural [t%128, tb, d]; big rearranged DMAs
                vt = v_pool.tile([128, NTBMAX, HD], bf16, tag="vt")
                if ttot % 128:
                    # zero last block before loads (NaN-safe padded tail)
                    nc.vector.memset(vt[:, ntb - 1, :], 0.0)

                def load_v_rows(t0, nrows, src, src_row0):
                    while nrows > 0 and t0 % 128:
                        seg = min(nrows, 128 - t0 % 128)
                        nc.gpsimd.dma_start(
                            vt[t0 % 128 : t0 % 128 + seg, t0 // 128, :],
                            src[src_row0 : src_row0 + seg, :],
                        )
                        t0 += seg
                        src_row0 += seg
                        nrows -= seg
                    nfull = (nrows // 128) * 128
                    if nfull:
                        nc.gpsimd.dma_start(
                            vt[:, t0 // 128 : t0 // 128 + nfull // 128, :],
                            src[src_row0 : src_row0 + nfull, :].rearrange(
                                "(tb p) d -> p tb d", p=128
                            ),
                        )
                        t0 += nfull
                        src_row0 += nfull
                        nrows -= nfull
                    if nrows:
                        nc.gpsimd.dma_start(
                            vt[:nrows, t0 // 128, :],
                            src[src_row0 : src_row0 + nrows, :],
                        )

                col = 0
                for start, n in sq["runs"]:
                    load_v_rows(col, n * PAGE, vch, start * PAGE)
                    col += n * PAGE
                if ctxp32 > ctxp and ctxp // 128 != ntb - 1:
                    # NaN-safe zeros for the gap rows
                    load_v_rows(ctxp, ctxp32 - ctxp, zz, 0)
                load_v_rows(ctxp32, S, vh, b * S)

                tiles[b] = ((kta, ktb), vt, qt, mask_all[:, b, :])

            def emit_compute(b):
                sq = seqs[b]
                ctxp32, ttot, ntb = sq["ctxp32"], sq["ttot"], sq["ntb"]
                tq, qmin = sq["tq"], sq["qmin"]
                (kta, ktb), vt, qt, mask_sb = tiles[b]

                # Interleaved per t-block: scores(tb) on PE while exp(tb-1)
                # runs on ACT, then PV(tb-1)+denom(tb-1) right behind it.
                ptt = pt_pool.tile([128, NTBMAX, SG], bf16, tag="pt")
                outt = psum_o.tile([128, SG], f32, tag="outt")
                dent = psum_d.tile([128, SG], f32, tag="dent")
                chunks = ((0, 4), (4, 8))
                last_tb = [0, 0]
                for tb in range(ntb):
                    for ci, (g0, g1) in enumerate(chunks):
                        if max(qmin[tb], g0) < g1:
                            last_tb[ci] = tb

                def emit_scores(tb0, ng):
                    # ng t-blocks (1 or 2) share one psum tile + one exp call
                    qm = qmin[tb0]
                    s_ps = psum_s.tile([128, ng * SG], f32, tag="s")
                    for j in range(ng):
                        for c0, c1 in ((qm * 128, 512), (max(512, qm * 128), SG)):
                            if c0 >= c1:
                                continue
                            tb = tb0 + j
                            lt = (
                                kta[:, tb * 128 : (tb + 1) * 128]
                                if tb < 8
                                else ktb[:, tb * 128 - 1024 : (tb + 1) * 128 - 1024]
                            )
                            nc.tensor.matmul(
                                s_ps[:, j * SG + c0 : j * SG + c1],
                                lhsT=lt,
                                rhs=qt[:, c0:c1],
                                start=True,
                                stop=True,
                            )
                    if ng == 2:
                        assert qm == 0
                        nc.scalar.activation(
                            out=ptt[:, tb0 : tb0 + 2, :],
                            in_=s_ps,
                            func=mybir.ActivationFunctionType.Exp,
                            scale=SCALE,
                        )
                    else:
                        nc.scalar.activation(
                            out=ptt[:, tb0, qm * 128 :],
                            in_=s_ps[:, qm * 128 : SG],
                            func=mybir.ActivationFunctionType.Exp,
                            scale=SCALE,
                            bias=(
                                0.0
                                if sq["valid"][tb0]
                                else mask_sb[:, tb0 : tb0 + 1]
                            ),
                        )
                    # staircase: zero P^T rows of new-token blocks for
                    # earlier q-tiles inside this t-block's suffix
                    for tb in range(tb0, tb0 + ng):
                        for r0 in range(0, 128, 32):
                            t0 = tb * 128 + r0
                            if t0 < ctxp32 or t0 >= ttot:
                                continue
                            blk = (t0 - ctxp32) // 32
                            if blk > qmin[tb]:
                                nc.vector.memset(
                                    ptt[
                                        r0 : r0 + 32, tb, qmin[tb] * 128 : blk * 128
                                    ],
                                    0.0,
                                )

                def emit_pv(tb):
                    for ci, (g0, g1) in enumerate(chunks):
                        lo = max(qmin[tb], g0)
                        if lo >= g1:
                            continue
                        nc.tensor.matmul(
                            outt[:, lo * 128 : g1 * 128],
                            lhsT=vt[:, tb, :],
                            rhs=ptt[:, tb, lo * 128 : g1 * 128],
                            start=(tb == 0),
                            stop=(tb == last_tb[ci]),
                        )
                    for ci, (g0, g1) in enumerate(chunks):
                        lo = max(qmin[tb], g0)
                        if lo >= g1:
                            continue
                        nc.tensor.matmul(
                            dent[:, lo * 128 : g1 * 128],
                            lhsT=ones_t,
                            rhs=ptt[:, tb, lo * 128 : g1 * 128],
                            start=(tb == 0),
                            stop=(tb == last_tb[ci]),
                        )

                pending = []
                for tb0, ng in sq["groups"]:
                    emit_scores(tb0, ng)
                    for tb in pending:
                        emit_pv(tb)
                    pending = list(range(tb0, tb0 + ng))
                for tb in pending:
                    emit_pv(tb)
                tiles[b] = (outt, dent)

            def emit_endgame(b):
                outt, dent = tiles.pop(b)
                # OUT^T * (1/denom) -> fp32 -> HBM (host reindexes [d,sg])
                invt = out_pool.tile([128, SG], f32, tag="invt")
                nc.vector.reciprocal_approx_fast(invt, dent)
                otf = out_pool.tile([128, SG], f32, tag="otf")
                nc.vector.tensor_mul(otf, outt, invt)
                nc.gpsimd.dma_start(outh[b], otf)

            # software-pipelined emission: the in-order SP/Pool sequencers
            # must issue seq b+2's loads before blocking on seq b's endgame.
            # Process largest seqs first: their long compute covers the
            # load latency of everything behind them.
            order = sorted(range(B), key=lambda b: -seqs[b]["ntb"])
            emit_loads(order[0])
            emit_loads(order[1])
            for j, b in enumerate(order):
                emit_compute(b)
                if j + 2 < B:
                    emit_loads(order[j + 2])
                emit_endgame(b)
    return nc


def _compile(seqs):
    import concourse.bacc as bacc

    nc = bacc.Bacc(
        "TRN2",
        target_bir_lowering=False,
        debug=False,
        enable_asserts=False,
        num_devices=8,
    )
    _build(nc, seqs)
    nc.compile()
    return nc


def kernel(q, k, v, k_cache, v_cache, page_tables, context_lens, page_size, block_size, **_):
    from concourse import bass_utils

    q = np.asarray(q)
    k = np.asarray(k)
    v = np.asarray(v)
    k_cache = np.asarray(k_cache)
    v_cache = np.asarray(v_cache)
    page_tables = np.asarray(page_tables)
    context_lens = np.asarray(context_lens)
    assert int(page_size) == PAGE and int(block_size) == BLOCK
    assert q.shape == (B * S, NUM_HEADS * HD)
    assert page_tables.shape == (B, MAX_PAGES)

    seqs = _schedule(page_tables, context_lens)
    nc = _compile(seqs)

    bf = ml_dtypes.bfloat16
    masks = _masks(seqs)
    kcv = k_cache.reshape(MAX_PAGES * B * PAGE, NUM_KV_HEADS, HD)
    vcv = v_cache.reshape(MAX_PAGES * B * PAGE, NUM_KV_HEADS, HD)
    zz = np.zeros((32, HD), bf)
    in_maps = []
    for n in range(NUM_KV_HEADS):
        in_maps.append(
            {
                "qh": np.ascontiguousarray(
                    q[:, n * G * HD : (n + 1) * G * HD]
                ).astype(bf),
                "kh": np.ascontiguousarray(k[:, n * HD : (n + 1) * HD]).astype(bf),
                "vh": np.ascontiguousarray(v[:, n * HD : (n + 1) * HD]).astype(bf),
                "kch": np.ascontiguousarray(kcv[:, n, :]).astype(bf),
                "vch": np.ascontiguousarray(vcv[:, n, :]).astype(bf),
                "mh": masks,
                "zz": zz,
            }
        )

    res = bass_utils.run_bass_kernel_spmd(nc, in_maps, core_ids=list(range(8)))
    global _last_results
    _last_results = res
    # per-core outh is [B, HD, SG=(s,g)]; assemble [B*S, (n,g)*HD]
    out = np.empty((B * S, NUM_HEADS * HD), np.float32)
    ov = out.reshape(B, S, NUM_KV_HEADS, G, HD)
    for n in range(NUM_KV_HEADS):
        # [B, HD, S*G] -> [B, S, G, HD]
        on = res.results[n]["outh"].reshape(B, HD, S, G)
        ov[:, :, n, :, :] = on.transpose(0, 2, 3, 1)
    return out


_last_results = None
